# revision 5
# baseline (speedup 1.0000x reference)
"""Trainium2 Bass kernel for nn_ConsistLoss (retrieval_knn).

Math notes
----------
reference() = mean(|rigid_refine - pred^T|) where
  rigid_refine = rigid_recon - mean_i(laplace_x_i - laplace_y_i)
  laplace_c_i  = (sum_{j in 6NN_c(i)} c_j - 6*q_i) / 5       (c in {x=rigid_recon, y})
The -6*q_i terms cancel in (laplace_x - laplace_y), and only the MEAN over all
i is needed, so:
  mean_vec = ( sum_j cx(j)*x_j - sum_j cy(j)*y_j ) / (5*N)
where cx(j) = #queries having ref j among their 6 nearest (mask column sums).

Device work per core (512 queries x 4096 refs x 2 clouds):
  s[q,j] = 2*q.x_j - |x_j|^2  (= |q|^2 - dist2; row-constant shift is rank-safe)
  computed as one K=4 matmul with lhsT=[q^T; 1], rhs=[2X^T; -|x|^2].
  top-8 per row via DVE InstMax -> threshold t=6th largest; mask = (s >= t);
  column sums of mask via ones-matmul on PE. Host: Kabsch (3x3 SVD) + O(N) tail.
"""

import os
from contextlib import ExitStack

import numpy as np

import concourse.bass as bass  # noqa: F401  (AP types / plumbing)
import concourse.tile as tile
from concourse import bacc, mybir
from concourse.bass_utils import run_bass_kernel_spmd

N = 4096          # points per cloud
NCORES = 8
NQ = N // NCORES  # 512 queries per core
P = 128           # SBUF partitions
QT = NQ // P      # 4 query tiles per core
CHS = 512         # free-dim chunk = one fp32 PSUM bank
CH = N // CHS     # 8 chunks
L_K = 6

_cache = {}
last_results = None  # test harness reads exec_time_ns off this


def _build_bass():
    nc = bacc.Bacc(
        "TRN2", target_bir_lowering=False, debug=False, num_devices=NCORES
    )
    f32 = mybir.dt.float32
    qa_d = nc.dram_tensor("qa", [4, NQ], f32, kind="ExternalInput")
    rx_d = nc.dram_tensor("rx", [4, N], f32, kind="ExternalInput")
    ry_d = nc.dram_tensor("ry", [4, N], f32, kind="ExternalInput")
    cnt_d = nc.dram_tensor("cnt", [1, 2 * N], f32, kind="ExternalOutput")

    with ExitStack() as ctx:
        tc = ctx.enter_context(tile.TileContext(nc))
        const_pool = ctx.enter_context(tc.tile_pool(name="const", bufs=1))
        s_pool = ctx.enter_context(tc.tile_pool(name="s", bufs=2 * QT))
        t8_pool = ctx.enter_context(tc.tile_pool(name="t8", bufs=2 * QT))
        ps_pool = ctx.enter_context(tc.tile_pool(name="ps", bufs=4, space="PSUM"))
        cp_pool = ctx.enter_context(tc.tile_pool(name="cp", bufs=2, space="PSUM"))

        qa = const_pool.tile([4, NQ], f32)
        nc.sync.dma_start(qa[:], qa_d.ap())
        rx = const_pool.tile([4, N], f32)
        nc.sync.dma_start(rx[:], rx_d.ap())
        ry = const_pool.tile([4, N], f32)
        nc.sync.dma_start(ry[:], ry_d.ap())
        ones = const_pool.tile([P, 1], f32)
        nc.vector.memset(ones[:], 1.0)
        out_sb = const_pool.tile([1, 2 * N], f32)

        for ci, r in enumerate((rx, ry)):
            masks = []
            for qt in range(QT):
                s = s_pool.tile([P, N], f32, tag="s")
                for ch in range(CH):
                    ps = ps_pool.tile([P, CHS], f32, tag="ps")
                    nc.tensor.matmul(
                        ps[:],
                        qa[:, qt * P : (qt + 1) * P],
                        r[:, ch * CHS : (ch + 1) * CHS],
                        start=True,
                        stop=True,
                    )
                    nc.scalar.copy(s[:, ch * CHS : (ch + 1) * CHS], ps[:])
                t8 = t8_pool.tile([P, 8], f32, tag="t8")
                nc.vector.max(t8[:], s[:])
                # mask = (s >= 6th-largest), written in place over s
                for ch in range(CH):
                    nc.vector.tensor_scalar(
                        s[:, ch * CHS : (ch + 1) * CHS],
                        s[:, ch * CHS : (ch + 1) * CHS],
                        t8[:, 5:6],
                        None,
                        mybir.AluOpType.is_ge,
                    )
                masks.append(s)
            # column sums: cnt[ci, j] = #queries of this core with j in their 6NN
            for ch in range(CH):
                cp = cp_pool.tile([1, CHS], f32, tag="cp")
                for qt in range(QT):
                    nc.tensor.matmul(
                        cp[:],
                        ones[:],
                        masks[qt][:, ch * CHS : (ch + 1) * CHS],
                        start=(qt == 0),
                        stop=(qt == QT - 1),
                    )
                nc.scalar.copy(
                    out_sb[0:1, ci * N + ch * CHS : ci * N + (ch + 1) * CHS], cp[:]
                )
        nc.sync.dma_start(cnt_d.ap(), out_sb[:])

    nc.compile()
    return nc


def _get_nc():
    if "nc" not in _cache:
        _cache["nc"] = _build_bass()
    return _cache["nc"]


def _kabsch_recon(input_t, sf_t):
    """Mirror reference's f32 Kabsch pipeline in numpy; returns rigid_recon [N,3]."""
    pc = np.ascontiguousarray(input_t[0].T.astype(np.float32))  # [N,3]
    recon = pc + np.ascontiguousarray(sf_t[0].T.astype(np.float32))
    cp = pc.mean(axis=0)
    cr = recon.mean(axis=0)
    H = (pc - cp).T @ (recon - cr)
    U, _, Vt = np.linalg.svd(H.astype(np.float64))
    d = np.sign(np.linalg.det(Vt.T @ U.T))
    R = Vt.T @ (np.array([1.0, 1.0, d])[:, None] * U.T)
    t = cr.astype(np.float64) - R @ cp.astype(np.float64)
    return (pc.astype(np.float64) @ R.T + t).astype(np.float32)


def kernel(input_t, sf_t, y1, pred):
    input_t = np.asarray(input_t, dtype=np.float32)
    sf_t = np.asarray(sf_t, dtype=np.float32)
    y1 = np.asarray(y1, dtype=np.float32)
    pred = np.asarray(pred, dtype=np.float32)

    X = _kabsch_recon(input_t, sf_t)                       # rigid_recon [N,3]
    Y = np.ascontiguousarray(y1[0].T.astype(np.float32))   # [N,3]

    nx = (X.astype(np.float32) ** 2).sum(axis=1, dtype=np.float32)
    ny = (Y.astype(np.float32) ** 2).sum(axis=1, dtype=np.float32)
    rx = np.concatenate([2.0 * X.T, -nx[None, :]], axis=0).astype(np.float32)
    ry = np.concatenate([2.0 * Y.T, -ny[None, :]], axis=0).astype(np.float32)

    in_maps = []
    for c in range(NCORES):
        q = X[c * NQ : (c + 1) * NQ]                       # [NQ,3]
        qa = np.concatenate(
            [q.T, np.ones((1, NQ), np.float32)], axis=0
        ).astype(np.float32)
        in_maps.append(
            {"qa": np.ascontiguousarray(qa), "rx": rx, "ry": ry}
        )

    nc = _get_nc()
    global last_results
    res = run_bass_kernel_spmd(nc, in_maps, core_ids=list(range(NCORES)))
    last_results = res

    cnt = np.stack([r["cnt"].reshape(2, N) for r in res.results])  # [8, 2, N]
    cx = cnt[:, 0, :].sum(axis=0).astype(np.float64)
    cy = cnt[:, 1, :].sum(axis=0).astype(np.float64)

    Sx = X.astype(np.float64).T @ cx                       # [3]
    Sy = Y.astype(np.float64).T @ cy
    mean_vec = ((Sx - Sy) / ((L_K - 1) * N)).astype(np.float32)

    rigid_refine = X - mean_vec[None, :]
    predT = np.ascontiguousarray(pred[0].T.astype(np.float32))
    loss = np.abs(rigid_refine.astype(np.float64) - predT.astype(np.float64)).mean()
    return np.float32(loss)
